# revision 1
# baseline (speedup 1.0000x reference)
"""AWQ quantized linear (nn_AWQLinear) on 8 Trainium2 NeuronCores.

Computes y = (x / input_scale) @ W_hat.T + bias where
W_hat[o, k] = (q_weight[o, k] - 8) * scales[o, k // 32].

Strategy (tensor-parallel, column sharded): out_features (11008) are split
across 8 cores (1376 each, zero-padded to 1408 = 11*128). Each core
dequantizes its weight shard on device into fp16 (folding r = 1/input_scale
into the weights), then computes a [4096 tokens x 4096] @ [4096 x 1408]
matmul in fp16 with fp32 PSUM accumulation, streaming the activations.
Host does layout-only prep (transpose / slice / pad / repeat).
"""
import sys
from contextlib import ExitStack

import numpy as np

sys.path.insert(0, "/opt/trn_rl_repo")

import concourse.bass as bass  # noqa: E402
import concourse.tile as tile  # noqa: E402
from concourse import bacc, mybir  # noqa: E402
from concourse.bass_utils import run_bass_kernel_spmd  # noqa: E402

# Problem constants (hardcoded per contest contract)
B, S, IN_F, OUT_F, BLOCK = 2, 2048, 4096, 11008, 32
N_CORES = 8
T = B * S            # 4096 tokens
K = IN_F             # 4096 contraction
OPC = OUT_F // N_CORES  # 1376 true out features per core
O = ((OPC + 127) // 128) * 128  # 1408 padded
T_CHUNK = 512

FP = mybir.dt.float16  # matmul operand dtype


def _build_nc():
    nkt = K // 128
    nc = bacc.Bacc(
        "TRN2",
        target_bir_lowering=False,
        debug=False,
        enable_asserts=False,
        num_devices=N_CORES,
    )
    xT = nc.dram_tensor("xT", [K, T], mybir.dt.float32, kind="ExternalInput").ap()
    q8T = nc.dram_tensor("q8T", [K, O], mybir.dt.int8, kind="ExternalInput").ap()
    sclT = nc.dram_tensor("sclT", [K, O], mybir.dt.float32, kind="ExternalInput").ap()
    rcol = nc.dram_tensor(
        "rcol", [128, nkt], mybir.dt.float32, kind="ExternalInput"
    ).ap()
    biasrow = nc.dram_tensor(
        "biasrow", [1, O], mybir.dt.float32, kind="ExternalInput"
    ).ap()
    out = nc.dram_tensor("out", [T, O], mybir.dt.float32, kind="ExternalOutput").ap()

    # output free-dim chunks, each <= 512 (one PSUM bank)
    ochunks = []
    o0 = 0
    while o0 < O:
        nn = min(512, O - o0)
        ochunks.append((o0, nn))
        o0 += nn

    with tile.TileContext(nc) as tc, ExitStack() as ctx:
        const_pool = ctx.enter_context(tc.tile_pool(name="const", bufs=1))
        w2_pool = ctx.enter_context(tc.tile_pool(name="w2", bufs=1))
        wstage = ctx.enter_context(tc.tile_pool(name="wstage", bufs=2))
        xs_pool = ctx.enter_context(tc.tile_pool(name="xs", bufs=2))
        out_pool = ctx.enter_context(tc.tile_pool(name="outp", bufs=2))
        psum_pool = ctx.enter_context(tc.tile_pool(name="psum", bufs=2, space="PSUM"))

        # prep: r = 1/input_scale laid out [128, nkt]; bias broadcast row
        rcol_sb = const_pool.tile([128, nkt], mybir.dt.float32)
        nc.sync.dma_start(rcol_sb[:], rcol[:])
        r_sb = const_pool.tile([128, nkt], mybir.dt.float32)
        nc.vector.reciprocal(r_sb[:], rcol_sb[:])

        brow_sb = const_pool.tile([1, O], mybir.dt.float32)
        nc.sync.dma_start(brow_sb[:], biasrow[:])
        bias_bc = const_pool.tile([128, O], mybir.dt.float32)
        nc.gpsimd.partition_broadcast(bias_bc[:], brow_sb[:])

        # phase 0: dequantize weights into resident fp16 W2[k, o]
        w2_all = w2_pool.tile([128, nkt, O], FP)
        for kt in range(nkt):
            q_t = wstage.tile([128, O], mybir.dt.int8, tag="qstage")
            nc.sync.dma_start(q_t[:], q8T[kt * 128 : (kt + 1) * 128, :])
            s_t = wstage.tile([128, O], mybir.dt.float32, tag="sstage")
            nc.sync.dma_start(s_t[:], sclT[kt * 128 : (kt + 1) * 128, :])
            t1 = wstage.tile([128, O], FP, tag="t1stage")
            nc.vector.tensor_scalar(
                t1[:], q_t[:], -8.0, r_sb[:, kt : kt + 1],
                op0=mybir.AluOpType.add, op1=mybir.AluOpType.mult,
            )
            nc.vector.tensor_tensor(
                w2_all[:, kt, :], t1[:], s_t[:], op=mybir.AluOpType.mult
            )

        # main loop: stream x chunks (cast fp32->fp16 in DMA), matmul, drain
        n_chunks = T // T_CHUNK
        tsub_per_chunk = T_CHUNK // 128
        for tc_i in range(n_chunks):
            xs_c = xs_pool.tile([128, nkt, T_CHUNK], FP)
            src = xT.rearrange("(kt p) t -> p kt t", p=128)[
                :, :, tc_i * T_CHUNK : (tc_i + 1) * T_CHUNK
            ]
            nc.gpsimd.dma_start(xs_c[:], src)

            for tsub in range(tsub_per_chunk):
                tt = tc_i * tsub_per_chunk + tsub
                out_sb = out_pool.tile([128, O], mybir.dt.float32, tag="osb")
                for oc, (o0, nn) in enumerate(ochunks):
                    ps = psum_pool.tile([128, nn], mybir.dt.float32, tag=f"ps{oc}")
                    for kt in range(nkt):
                        lhsT = xs_c[:, kt, tsub * 128 : (tsub + 1) * 128]
                        nc.tensor.matmul(
                            ps[:],
                            lhsT,
                            w2_all[:, kt, o0 : o0 + nn],
                            start=(kt == 0),
                            stop=(kt == nkt - 1),
                        )
                    nc.vector.tensor_tensor(
                        out_sb[:, o0 : o0 + nn], ps[:], bias_bc[:, o0 : o0 + nn],
                        op=mybir.AluOpType.add,
                    )
                nc.sync.dma_start(out[tt * 128 : (tt + 1) * 128, :], out_sb[:])

    nc.compile()
    return nc


_NC_CACHE = None


def _get_nc():
    global _NC_CACHE
    if _NC_CACHE is None:
        _NC_CACHE = _build_nc()
    return _NC_CACHE


def _host_prepare(x, q_weight, scales, input_scale, bias):
    """Layout-only host prep (transpose / slice / pad / repeat / repack)."""
    xT = np.ascontiguousarray(np.asarray(x, np.float32).reshape(T, K).T)
    rcol = np.ascontiguousarray(
        np.asarray(input_scale, np.float32).reshape(K // 128, 128).T
    )
    q8 = np.asarray(q_weight).astype(np.int8)  # lossless repack of codes 0..15
    scales = np.asarray(scales, np.float32)
    bias = np.asarray(bias, np.float32)

    in_maps = []
    for c in range(N_CORES):
        rows = slice(c * OPC, (c + 1) * OPC)
        qc, sc, bc = q8[rows], scales[rows], bias[rows]
        pad = O - OPC
        if pad:
            qc = np.concatenate([qc, np.zeros((pad, K), np.int8)], axis=0)
            sc = np.concatenate([sc, np.zeros((pad, K // BLOCK), np.float32)], axis=0)
            bc = np.concatenate([bc, np.zeros((pad,), np.float32)], axis=0)
        in_maps.append(
            {
                "xT": xT,
                "q8T": np.ascontiguousarray(qc.T),
                "sclT": np.ascontiguousarray(np.repeat(sc.T, BLOCK, axis=0)),
                "rcol": rcol,
                "biasrow": np.ascontiguousarray(bc[None, :]),
            }
        )
    return in_maps


def _run(inputs, trace=False, **kw):
    in_maps = _host_prepare(**inputs)
    nc = _get_nc()
    res = run_bass_kernel_spmd(
        nc, in_maps, core_ids=list(range(N_CORES)), trace=trace, **kw
    )
    parts = [r["out"][:, :OPC] for r in res.results]
    full = np.concatenate(parts, axis=1).reshape(B, S, OUT_F).astype(np.float32)
    return full, res


def kernel(x, q_weight, scales, input_scale, bias):
    out, _ = _run(
        dict(x=x, q_weight=q_weight, scales=scales,
             input_scale=input_scale, bias=bias)
    )
    return out
